# revision 5
# baseline (speedup 1.0000x reference)
"""Trainium2 Bass kernel for a transformer decoder layer (self-attn + cross-attn).

Sharding: 8 cores = 4 batches x 2 head-groups (8 heads each).
Each core computes, for its batch b and its 8 heads:
  - LN1 (full batch rows, duplicated within the pair)
  - Q1/K1/V1 projections (column-sharded by head), causal self-attention,
    O1 projection (row-sharded -> partial sums)
  - pairwise AllReduce of the O1 partials -> x1 = x + sa_out
  - LN2, Q2/K2/V2 (cross) projections, cross-attention (writes its 8 heads of
    ca_w), O2 projection (partial)
  - outputs x2_partial = 0.5*x1 + ca_partial; host sums the pair.

Layouts: activations for matmul inputs are kept transposed ([feature, token]),
scores are computed as S^T = K @ Q^T ([key, row]) so the key-padding mask is a
per-partition exp bias and softmax sums come free from an appended ones column
in V. P~ for cross-attention is PE-transposed back to [row, key] with the
per-row 1/sum normalization fused into the PSUM eviction, so ca_w stores are
contiguous.
"""

import numpy as np

B, T, S, D, H = 4, 1024, 1024, 1024, 16
DH = 64
HL = 8            # heads per core
EPS = 1e-5
RB = 256          # row block
NRB = 4
KCW = 128         # key chunk width
NKC = 8
NDC = 8           # D chunks (contraction)
NEG = -1e30

_PROG = None


def _build_program():
    import concourse.bacc as bacc
    import concourse.mybir as mybir
    import concourse.tile as tile
    from contextlib import ExitStack

    F32 = mybir.dt.float32
    F32R = mybir.dt.float32r
    AF = mybir.ActivationFunctionType
    ALU = mybir.AluOpType

    nc = bacc.Bacc(None, target_bir_lowering=False)

    # ---- dram I/O ----
    x_in = nc.dram_tensor("x_in", [T, D], F32, kind="ExternalInput")
    enc_in = nc.dram_tensor("enc_in", [S, D], F32R, kind="ExternalInput")
    wq1 = nc.dram_tensor("wq1", [128, NDC, 512], F32R, kind="ExternalInput")
    wk1 = nc.dram_tensor("wk1", [128, NDC, 512], F32R, kind="ExternalInput")
    wv1 = nc.dram_tensor("wv1", [128, NDC, 520], F32R, kind="ExternalInput")
    wo1 = nc.dram_tensor("wo1", [128, 4, 1024], F32R, kind="ExternalInput")
    wq2 = nc.dram_tensor("wq2", [128, NDC, 512], F32R, kind="ExternalInput")
    wk2 = nc.dram_tensor("wk2", [128, NDC, 512], F32R, kind="ExternalInput")
    wv2 = nc.dram_tensor("wv2", [128, NDC, 520], F32R, kind="ExternalInput")
    wo2 = nc.dram_tensor("wo2", [128, 4, 1024], F32R, kind="ExternalInput")
    bq1 = nc.dram_tensor("bq1", [128, 4], F32, kind="ExternalInput")
    bk1 = nc.dram_tensor("bk1", [128, 4], F32, kind="ExternalInput")
    bv1 = nc.dram_tensor("bv1", [1, 520], F32R, kind="ExternalInput")
    bo1 = nc.dram_tensor("bo1", [1, 1024], F32R, kind="ExternalInput")
    bq2 = nc.dram_tensor("bq2", [128, 4], F32, kind="ExternalInput")
    bk2 = nc.dram_tensor("bk2", [128, 4], F32, kind="ExternalInput")
    bv2 = nc.dram_tensor("bv2", [1, 520], F32R, kind="ExternalInput")
    bo2 = nc.dram_tensor("bo2", [1, 1024], F32R, kind="ExternalInput")
    selfpad = nc.dram_tensor("selfpad", [128, NKC], F32, kind="ExternalInput")
    crosspad = nc.dram_tensor("crosspad", [128, NKC], F32, kind="ExternalInput")
    onesr = nc.dram_tensor("onesr", [1, 128], F32R, kind="ExternalInput")
    eyer = nc.dram_tensor("eyer", [128, 128], F32R, kind="ExternalInput")
    eyef = nc.dram_tensor("eyef", [1, 1], F32, kind="ExternalInput")

    xo = nc.dram_tensor("xo", [T, D], F32, kind="ExternalOutput")
    ca = nc.dram_tensor("ca", [HL, T, S], F32, kind="ExternalOutput")

    cc_in = nc.dram_tensor("cc_in", [T, D], F32)
    cc_out = nc.dram_tensor("cc_out", [T, D], F32)
    groups = [[0, 1], [2, 3], [4, 5], [6, 7]]

    with tile.TileContext(nc) as tc, ExitStack() as top:
        const = top.enter_context(tc.tile_pool(name="const", bufs=1))
        t_eyer = const.tile([128, 128], F32R)
        nc.sync.dma_start(out=t_eyer, in_=eyer[:])
        t_eyef = const.tile([1, 1], F32)
        nc.sync.dma_start(out=t_eyef, in_=eyef[:])
        t_ones = const.tile([1, 128], F32R)
        nc.sync.dma_start(out=t_ones, in_=onesr[:])
        t_spad = const.tile([128, NKC], F32)
        nc.sync.dma_start(out=t_spad, in_=selfpad[:])
        t_cpad = const.tile([128, NKC], F32)
        nc.sync.dma_start(out=t_cpad, in_=crosspad[:])
        t_eps = const.tile([128, 1], F32)
        nc.vector.memset(t_eps, EPS)
        t_bq1 = const.tile([128, 4], F32); nc.sync.dma_start(out=t_bq1, in_=bq1[:])
        t_bk1 = const.tile([128, 4], F32); nc.sync.dma_start(out=t_bk1, in_=bk1[:])
        t_bq2 = const.tile([128, 4], F32); nc.sync.dma_start(out=t_bq2, in_=bq2[:])
        t_bk2 = const.tile([128, 4], F32); nc.sync.dma_start(out=t_bk2, in_=bk2[:])
        t_bv1 = const.tile([1, 520], F32R); nc.sync.dma_start(out=t_bv1, in_=bv1[:])
        t_bo1 = const.tile([1, 1024], F32R); nc.sync.dma_start(out=t_bo1, in_=bo1[:])
        t_bv2 = const.tile([1, 520], F32R); nc.sync.dma_start(out=t_bv2, in_=bv2[:])
        t_bo2 = const.tile([1, 1024], F32R); nc.sync.dma_start(out=t_bo2, in_=bo2[:])

        xpool = top.enter_context(tc.tile_pool(name="xpool", bufs=1))
        x_t = xpool.tile([128, 8, 1024], F32)   # token = tc*128 + p
        nc.sync.dma_start(out=x_t, in_=x_in.rearrange("(t p) d -> p t d", p=128))

        wpool = top.enter_context(tc.tile_pool(name="wpool", bufs=2))
        encT_pool = top.enter_context(tc.tile_pool(name="encT", bufs=1))
        encT = encT_pool.tile([128, NDC, S], F32R)

        # ---------- helpers ----------
        def layernorm_chunk(lnp, psp, src_ap, dst_f32r_tile):
            """LN over free dim of a [128,1024] f32 chunk -> writes f32r tile."""
            stats = lnp.tile([128, 2, 6], F32, tag="stats")
            nc.vector.bn_stats(out=stats[:, 0], in_=src_ap[:, 0:512])
            nc.vector.bn_stats(out=stats[:, 1], in_=src_ap[:, 512:1024])
            mv = lnp.tile([128, 2], F32, tag="mv")
            nc.vector.bn_aggr(out=mv[:], in_=stats[:])
            lnv = lnp.tile([128, 1], F32, tag="lnv")
            nc.scalar.activation(out=lnv[:], in_=mv[:, 1:2], func=AF.Ln,
                                 bias=t_eps[:], scale=1.0)
            rstd = lnp.tile([128, 1], F32, tag="rstd")
            nc.scalar.activation(out=rstd[:], in_=lnv[:], func=AF.Exp,
                                 bias=0.0, scale=-0.5)
            nc.vector.tensor_scalar(out=dst_f32r_tile[:], in0=src_ap,
                                    scalar1=mv[:, 0:1], scalar2=rstd[:],
                                    op0=ALU.subtract, op1=ALU.mult)

        def transpose_chunk(psp, src_f32r_ap, dst_tile, dst_col0):
            """[128,1024] f32r normal chunk -> 8 [128,128] transposes into
            dst_tile[:, dc, dst_col0:dst_col0+128]."""
            for dc in range(NDC):
                ptr = psp.tile([128, 128], F32R, tag="ptr")
                nc.tensor.transpose(ptr[:], src_f32r_ap[:, dc * 128:(dc + 1) * 128],
                                    t_eyer[:])
                nc.scalar.activation(out=dst_tile[:, dc, dst_col0:dst_col0 + 128],
                                     in_=ptr[:], func=AF.Copy)

        def proj_T(psp, srcT, w_dram, bias_tile, out_tile):
            """out_tile [128, 4, 1024] f32r: per hg,rcol accumulate 8 dchunks."""
            wt = wpool.tile([128, NDC, 512], F32R, tag="w")
            nc.sync.dma_start(out=wt, in_=w_dram[:])
            for hg in range(4):
                for rc2 in range(2):
                    pp = psp.tile([128, 512], F32, tag="pproj")
                    for dc in range(NDC):
                        nc.tensor.matmul(
                            pp[:], lhsT=wt[:, dc, hg * 128:(hg + 1) * 128],
                            rhs=srcT[:, dc, rc2 * 512:(rc2 + 1) * 512],
                            start=(dc == 0), stop=(dc == NDC - 1))
                    nc.scalar.activation(
                        out=out_tile[:, hg, rc2 * 512:(rc2 + 1) * 512], in_=pp[:],
                        func=AF.Identity, bias=bias_tile[:, hg:hg + 1], scale=1.0)

        def proj_V(psp, srcT, w_dram, brow_tile, out_tile):
            """V' projection: out_tile [128, NKC, 520] f32r."""
            wt = wpool.tile([128, NDC, 520], F32R, tag="w")
            nc.sync.dma_start(out=wt, in_=w_dram[:])
            for kc in range(NKC):
                for g4 in range(2):
                    pp = psp.tile([128, 260], F32, tag="pproj")
                    for dc in range(NDC):
                        nc.tensor.matmul(
                            pp[:], lhsT=srcT[:, dc, kc * 128:(kc + 1) * 128],
                            rhs=wt[:, dc, g4 * 260:(g4 + 1) * 260],
                            start=(dc == 0), stop=False)
                    nc.tensor.matmul(pp[:], lhsT=t_ones[:, 0:128],
                                     rhs=brow_tile[:, g4 * 260:(g4 + 1) * 260],
                                     start=False, stop=True)
                    nc.vector.tensor_copy(
                        out=out_tile[:, kc, g4 * 260:(g4 + 1) * 260], in_=pp[:])

        def proj_O(psp, ctxT, w_dram, brow_tile, evict_fn):
            """O projection: for each row chunk rc, produce psum [128,1024] and
            call evict_fn(rc, psum_ap)."""
            wt = wpool.tile([128, 4, 1024], F32R, tag="w")
            nc.sync.dma_start(out=wt, in_=w_dram[:])
            for rc in range(8):
                po = psp.tile([128, 2, 512], F32, tag="po")
                for nh in range(2):
                    for cc in range(4):
                        nc.tensor.matmul(
                            po[:, nh], lhsT=ctxT[:, cc, rc * 128:(rc + 1) * 128],
                            rhs=wt[:, cc, nh * 512:(nh + 1) * 512],
                            start=(cc == 0), stop=False)
                    nc.tensor.matmul(po[:, nh], lhsT=t_ones[:, 0:128],
                                     rhs=brow_tile[:, nh * 512:(nh + 1) * 512],
                                     start=False, stop=True)
                evict_fn(rc, po)

        # ---------- phase L1 + transpose, and encT ----------
        es_a = ExitStack()
        qkv1_pool = es_a.enter_context(tc.tile_pool(name="qkv1", bufs=1))
        q1T = qkv1_pool.tile([128, 4, 1024], F32R)
        k1T = qkv1_pool.tile([128, 4, 1024], F32R)
        v1 = qkv1_pool.tile([128, NKC, 520], F32R)

        with tc.tile_pool(name="xnT", bufs=1, side="right") as xnT_pool, \
             tc.tile_pool(name="lnp", bufs=2) as lnp, \
             tc.tile_pool(name="psL1", bufs=4, space="PSUM") as psL1, \
             tc.tile_pool(name="psP1", bufs=3, space="PSUM") as psP1:
            xnT = xnT_pool.tile([128, NDC, T], F32R)
            for tc8 in range(8):
                xn = lnp.tile([128, 1024], F32R, tag="xn")
                layernorm_chunk(lnp, psL1, x_t[:, tc8, :], xn)
                transpose_chunk(psL1, xn[:], xnT, tc8 * 128)
            # encoder transpose (enc already f32r in dram)
            for tc8 in range(8):
                ec = lnp.tile([128, 1024], F32R, tag="xn")
                nc.sync.dma_start(
                    out=ec, in_=enc_in.rearrange("(t p) d -> p t d", p=128)[:, tc8, :])
                transpose_chunk(psL1, ec[:], encT, tc8 * 128)

            proj_T(psP1, xnT, wq1, t_bq1, q1T)
            proj_T(psP1, xnT, wk1, t_bk1, k1T)
            proj_V(psP1, xnT, wv1, t_bv1, v1)

        # ---------- phase A1: self attention ----------
        es_b = ExitStack()
        ctx1_pool = es_b.enter_context(tc.tile_pool(name="ctx1", bufs=1, side="right"))
        ctx1T = ctx1_pool.tile([128, 4, 1024], F32R)
        with tc.tile_pool(name="pt1", bufs=2) as pt1_pool, \
             tc.tile_pool(name="att_sm", bufs=2) as smp, \
             tc.tile_pool(name="psS", bufs=3, space="PSUM") as psS, \
             tc.tile_pool(name="psC", bufs=2, space="PSUM") as psC:
            for h in range(HL):
                po, pg = (h % 2) * 64, h // 2
                for rb in range(NRB):
                    nkc = 2 * (rb + 1)
                    ptil = pt1_pool.tile([128, NKC, RB], F32R, tag="pt")
                    for kc in range(nkc):
                        ps_s = psS.tile([128, RB], F32, tag="ps_s")
                        nc.tensor.matmul(
                            ps_s[:],
                            lhsT=k1T[po:po + 64, pg, kc * 128:(kc + 1) * 128],
                            rhs=q1T[po:po + 64, pg, rb * RB:(rb + 1) * RB],
                            start=True, stop=True)
                        nc.scalar.activation(
                            out=ptil[:, kc], in_=ps_s[:], func=AF.Exp,
                            bias=t_spad[:, kc:kc + 1], scale=0.125)
                        if kc >= 2 * rb:
                            nc.gpsimd.affine_select(
                                out=ptil[:, kc], in_=ptil[:, kc],
                                compare_op=ALU.is_ge, fill=0.0,
                                base=rb * RB - kc * 128,
                                channel_multiplier=-1, pattern=[[1, RB]])
                    ps_c = psC.tile([65, RB], F32, tag="ps_c")
                    for kc in range(nkc):
                        nc.tensor.matmul(ps_c[:],
                                         lhsT=v1[:, kc, h * 65:(h + 1) * 65],
                                         rhs=ptil[:, kc],
                                         start=(kc == 0), stop=(kc == nkc - 1))
                    rsb = smp.tile([1, RB], F32, tag="rsb")
                    nc.vector.reciprocal(out=rsb[:], in_=ps_c[64:65, :])
                    rbc = smp.tile([64, RB], F32, tag="rbc")
                    nc.gpsimd.partition_broadcast(rbc[:], rsb[:])
                    nc.vector.tensor_mul(
                        out=ctx1T[po:po + 64, pg, rb * RB:(rb + 1) * RB],
                        in0=ps_c[0:64, :], in1=rbc[:])

        # ---------- phase O1 + AllReduce + L2 ----------
        es_a.close()  # free qkv1
        with tc.tile_pool(name="sa_sb", bufs=3) as sa_pool, \
             tc.tile_pool(name="psO", bufs=2, space="PSUM") as psO:
            def evict_sa(rc, po):
                sa = sa_pool.tile([128, 1024], F32, tag="sa")
                nc.scalar.activation(out=sa[:], in_=po[:].rearrange("p a b -> p (a b)"),
                                     func=AF.Copy)
                nc.sync.dma_start(out=cc_in[rc * 128:(rc + 1) * 128, :], in_=sa[:])
            proj_O(psO, ctx1T, wo1, t_bo1, evict_sa)

        for half in range(2):
            nc.gpsimd.collective_compute(
                kind="AllReduce", op=mybir.AluOpType.add,
                ins=[cc_in[half * 512:(half + 1) * 512, :]],
                outs=[cc_out[half * 512:(half + 1) * 512, :]],
                replica_groups=groups)

        es_b.close()  # free ctx1
        es_c = ExitStack()
        qkv2_pool = es_c.enter_context(tc.tile_pool(name="qkv2", bufs=1))
        q2T = qkv2_pool.tile([128, 4, 1024], F32R)
        k2T = qkv2_pool.tile([128, 4, 1024], F32R)
        v2 = qkv2_pool.tile([128, NKC, 520], F32R)

        with tc.tile_pool(name="xn2T", bufs=1, side="right") as xn2T_pool, \
             tc.tile_pool(name="lnp2", bufs=2) as lnp2, \
             tc.tile_pool(name="psL2", bufs=4, space="PSUM") as psL2, \
             tc.tile_pool(name="psP2", bufs=3, space="PSUM") as psP2:
            xn2T = xn2T_pool.tile([128, NDC, T], F32R)
            for tc8 in range(8):
                sa_ch = lnp2.tile([128, 1024], F32, tag="sach")
                nc.sync.dma_start(out=sa_ch,
                                  in_=cc_out[tc8 * 128:(tc8 + 1) * 128, :])
                nc.vector.tensor_add(out=x_t[:, tc8, :], in0=x_t[:, tc8, :],
                                     in1=sa_ch[:])
                xn2 = lnp2.tile([128, 1024], F32R, tag="xn2")
                layernorm_chunk(lnp2, psL2, x_t[:, tc8, :], xn2)
                transpose_chunk(psL2, xn2[:], xn2T, tc8 * 128)
            proj_T(psP2, xn2T, wq2, t_bq2, q2T)
            proj_T(psP2, encT, wk2, t_bk2, k2T)
            proj_V(psP2, encT, wv2, t_bv2, v2)

        # ---------- phase A2: cross attention ----------
        es_d = ExitStack()
        ctx2_pool = es_d.enter_context(tc.tile_pool(name="ctx2", bufs=1, side="right"))
        ctx2T = ctx2_pool.tile([128, 4, 1024], F32R)
        with tc.tile_pool(name="pt2", bufs=2) as pt2_pool, \
             tc.tile_pool(name="ca_sb", bufs=3) as ca_pool, \
             tc.tile_pool(name="att_sm2", bufs=2) as smp2, \
             tc.tile_pool(name="psS2", bufs=2, space="PSUM") as psS2, \
             tc.tile_pool(name="psC2", bufs=2, space="PSUM") as psC2, \
             tc.tile_pool(name="psT2", bufs=3, space="PSUM") as psT2, \
             tc.tile_pool(name="psR2", bufs=1, space="PSUM") as psR2:
            for h in range(HL):
                po, pg = (h % 2) * 64, h // 2
                for rb in range(NRB):
                    ptil = pt2_pool.tile([128, NKC, RB], F32R, tag="pt")
                    for kc in range(NKC):
                        ps_s = psS2.tile([128, RB], F32, tag="ps_s")
                        nc.tensor.matmul(
                            ps_s[:],
                            lhsT=k2T[po:po + 64, pg, kc * 128:(kc + 1) * 128],
                            rhs=q2T[po:po + 64, pg, rb * RB:(rb + 1) * RB],
                            start=True, stop=True)
                        nc.scalar.activation(
                            out=ptil[:, kc], in_=ps_s[:], func=AF.Exp,
                            bias=t_cpad[:, kc:kc + 1], scale=0.125)
                    ps_c = psC2.tile([65, RB], F32, tag="ps_c")
                    for kc in range(NKC):
                        nc.tensor.matmul(ps_c[:],
                                         lhsT=v2[:, kc, h * 65:(h + 1) * 65],
                                         rhs=ptil[:, kc],
                                         start=(kc == 0), stop=(kc == NKC - 1))
                    rsb = smp2.tile([1, RB], F32, tag="rsb")
                    nc.vector.reciprocal(out=rsb[:], in_=ps_c[64:65, :])
                    rbc = smp2.tile([64, RB], F32, tag="rbc")
                    nc.gpsimd.partition_broadcast(rbc[:], rsb[:])
                    nc.vector.tensor_mul(
                        out=ctx2T[po:po + 64, pg, rb * RB:(rb + 1) * RB],
                        in0=ps_c[0:64, :], in1=rbc[:])
                    # transpose recip -> per-row scale column [128, 2]
                    pr = psR2.tile([128, 2], F32, tag="pr")
                    nc.tensor.transpose(pr[:, 0:1], rsb[:, 0:128], t_eyef[:])
                    nc.tensor.transpose(pr[:, 1:2], rsb[:, 128:256], t_eyef[:])
                    rcol = smp2.tile([128, 2], F32, tag="rcol")
                    nc.scalar.activation(out=rcol[:], in_=pr[:], func=AF.Copy)
                    # transpose P~ back to [row, key] and evict normalized
                    for rh in range(2):
                        casb = ca_pool.tile([128, S], F32, tag="ca")
                        for kc2 in range(4):
                            ptr2 = psT2.tile([128, 2, 128], F32R, tag="ptr2")
                            for j in range(2):
                                kc = kc2 * 2 + j
                                nc.tensor.transpose(
                                    ptr2[:, j],
                                    ptil[:, kc, rh * 128:(rh + 1) * 128],
                                    t_eyer[:])
                            nc.scalar.activation(
                                out=casb[:, kc2 * 256:(kc2 + 1) * 256],
                                in_=ptr2[:].rearrange("p a b -> p (a b)"),
                                func=AF.Copy, scale=rcol[:, rh:rh + 1])
                        nc.sync.dma_start(
                            out=ca[h, rb * RB + rh * 128: rb * RB + (rh + 1) * 128, :],
                            in_=casb[:])

        # ---------- phase O2 + output ----------
        es_c.close()  # free qkv2
        with tc.tile_pool(name="xo_sb", bufs=3) as xo_pool, \
             tc.tile_pool(name="psO2", bufs=2, space="PSUM") as psO2:
            def evict_xo(rc, po):
                xov = xo_pool.tile([128, 1024], F32, tag="xo")
                nc.vector.tensor_scalar_mul(out=xov[:], in0=x_t[:, rc, :],
                                            scalar1=0.5)
                nc.vector.tensor_add(out=xov[:], in0=xov[:],
                                     in1=po[:].rearrange("p a b -> p (a b)"))
                nc.sync.dma_start(out=xo[rc * 128:(rc + 1) * 128, :], in_=xov[:])
            proj_O(psO2, ctx2T, wo2, t_bo2, evict_xo)
        es_d.close()

    nc.compile()
    return nc


def _prep_core_inputs(c, inp):
    """Host-side shard/prep for core c (b = c//2, head group g = c%2)."""
    b, g = c // 2, c % 2
    f32 = np.float32

    def fold_w(W, gvec):
        return (gvec[:, None] * W).astype(f32)

    def fold_b(W, bvec, beta):
        return (beta @ W + bvec).astype(f32)

    def wT_tiles(Wslice):          # [1024, 512] -> [128, 8, 512]
        return np.ascontiguousarray(
            Wslice.reshape(NDC, 128, Wslice.shape[1]).transpose(1, 0, 2)).astype(f32)

    def wV_tiles(Wslice, bslice):  # [1024,512],[512] -> [128,8,520],[1,520]
        Wp = np.zeros((D, HL, 65), f32)
        Wp[:, :, :64] = Wslice.reshape(D, HL, 64)
        bp = np.zeros((HL, 65), f32)
        bp[:, :64] = bslice.reshape(HL, 64)
        bp[:, 64] = 1.0
        return (np.ascontiguousarray(
                    Wp.reshape(D, 520).reshape(NDC, 128, 520).transpose(1, 0, 2)),
                bp.reshape(1, 520))

    sl = slice(g * 512, (g + 1) * 512)
    Wq1f = fold_w(inp["Wq1"], inp["ln1_g"]); bq1f = fold_b(inp["Wq1"], inp["bq1"], inp["ln1_b"])
    Wk1f = fold_w(inp["Wk1"], inp["ln1_g"]); bk1f = fold_b(inp["Wk1"], inp["bk1"], inp["ln1_b"])
    Wv1f = fold_w(inp["Wv1"], inp["ln1_g"]); bv1f = fold_b(inp["Wv1"], inp["bv1"], inp["ln1_b"])
    Wq2f = fold_w(inp["Wq2"], inp["ln2_g"]); bq2f = fold_b(inp["Wq2"], inp["bq2"], inp["ln2_b"])
    Wk2f = inp["Wk2"].astype(f32); bk2f = inp["bk2"].astype(f32)
    Wv2f = inp["Wv2"].astype(f32); bv2f = inp["bv2"].astype(f32)

    wv1t, bv1r = wV_tiles(Wv1f[:, sl], bv1f[sl])
    wv2t, bv2r = wV_tiles(Wv2f[:, sl], bv2f[sl])

    karr = np.arange(S).reshape(NKC, 128).T  # [128, NKC] global key index
    spad = np.where(karr < int(inp["tgt_lengths"][b]), 0.0, NEG).astype(f32)
    cpad = np.where(karr < int(inp["src_lengths"][b]), 0.0, NEG).astype(f32)

    return {
        "x_in": np.ascontiguousarray(inp["x"][b]).astype(f32),
        "enc_in": np.ascontiguousarray(inp["encoder_output"][b]).astype(f32),
        "wq1": wT_tiles(Wq1f[:, sl]), "wk1": wT_tiles(Wk1f[:, sl]), "wv1": wv1t,
        "wo1": np.ascontiguousarray(
            inp["Wo1"][sl, :].reshape(4, 128, 1024).transpose(1, 0, 2)).astype(f32),
        "wq2": wT_tiles(Wq2f[:, sl]), "wk2": wT_tiles(Wk2f[:, sl]), "wv2": wv2t,
        "wo2": np.ascontiguousarray(
            inp["Wo2"][sl, :].reshape(4, 128, 1024).transpose(1, 0, 2)).astype(f32),
        "bq1": bq1f[sl].reshape(4, 128).T.copy(),
        "bk1": bk1f[sl].reshape(4, 128).T.copy(),
        "bv1": bv1r, "bo1": (0.5 * inp["bo1"]).astype(f32).reshape(1, 1024),
        "bq2": bq2f[sl].reshape(4, 128).T.copy(),
        "bk2": bk2f[sl].reshape(4, 128).T.copy(),
        "bv2": bv2r, "bo2": (0.5 * inp["bo2"]).astype(f32).reshape(1, 1024),
        "selfpad": spad, "crosspad": cpad,
        "onesr": np.ones((1, 128), f32),
        "eyer": np.eye(128, dtype=f32),
        "eyef": np.ones((1, 1), f32),
    }


def kernel(**inputs):
    global _PROG
    from concourse.bass_utils import run_bass_kernel_spmd

    inp = {k: np.asarray(v) for k, v in inputs.items()}
    if _PROG is None:
        _PROG = _build_program()

    in_maps = [_prep_core_inputs(c, inp) for c in range(8)]
    res = run_bass_kernel_spmd(_PROG, in_maps, core_ids=list(range(8))).results

    x_out = np.empty((B, T, D), np.float32)
    ca_w = np.empty((B, H, T, S), np.float32)
    for b in range(B):
        x_out[b] = res[2 * b]["xo"] + res[2 * b + 1]["xo"]
        ca_w[b, 0:HL] = res[2 * b]["ca"]
        ca_w[b, HL:H] = res[2 * b + 1]["ca"]
    return x_out, ca_w


# revision 32
# speedup vs baseline: 1209.8936x; 1209.8936x over previous
"""Trainium2 Bass kernel for a transformer decoder layer (self-attn + cross-attn).

Sharding: 8 cores = 4 batches x 2 head-groups (8 heads each).
Each core computes, for its batch b and its 8 heads:
  - LN1 (full batch rows, duplicated within the pair)
  - Q1/K1/V1 projections (column-sharded by head), causal self-attention,
    O1 projection (row-sharded -> partial sums)
  - pairwise AllReduce of the O1 partials -> x1 = x + sa_out
  - LN2, Q2/K2/V2 (cross) projections, cross-attention (writes its 8 heads of
    ca_w), O2 projection (partial)
  - outputs x2_partial = 0.5*x1 + ca_partial; host sums the pair.

Layouts: activations for matmul inputs are kept transposed ([feature, token]),
scores are computed as S^T = K @ Q^T ([key, row]) so the key-padding mask is a
per-partition exp bias and softmax sums come free from an appended ones column
in V. P~ for cross-attention is PE-transposed back to [row, key] with the
per-row 1/sum normalization fused into the PSUM eviction, so ca_w stores are
contiguous.
"""

import numpy as np

B, T, S, D, H = 4, 1024, 1024, 1024, 16
DH = 64
HL = 8            # heads per core
EPS = 1e-5
RB = 512          # row block
NRB = 2
KCW = 128         # key chunk width
NKC = 8
NDC = 8           # D chunks (contraction)
NEG = -1e30

_PROG = None
PHASE_MARKS = {}
PIN_TABLES = True


def _build_program(fake_cc=False):
    import concourse.bacc as bacc
    import concourse.mybir as mybir
    import concourse.tile as tile
    from contextlib import ExitStack

    F32 = mybir.dt.float32
    F32R = mybir.dt.float32r
    AF = mybir.ActivationFunctionType
    ALU = mybir.AluOpType

    # Pin all activations to the one table set containing Exp/Ln/Copy/Abs so
    # the act-table-load pass emits a single load instead of thrashing sets.
    _orig_tables = bacc.get_activation_tables
    def _one_set(arch):
        # Keep the canonical set list and ordering (act_func_set_id indexes
        # act_info.json) but blank out every set except the one that holds
        # Exp/Ln/Copy/Abs, so the load-insertion pass maps all our
        # activations to a single table set (1 load, no thrash).
        t = dict(_orig_tables(arch))
        return {k: (v if k == "natural_log_exp_and_others" else set())
                for k, v in t.items()}
    if PIN_TABLES:
        bacc.get_activation_tables = _one_set

    nc = bacc.Bacc(None, target_bir_lowering=False)

    def mark(phase):
        PHASE_MARKS[phase] = int(nc.get_next_instruction_name().split("-")[1])

    # ---- dram I/O ----
    x_in = nc.dram_tensor("x_in", [T, D], F32, kind="ExternalInput")
    enc_in = nc.dram_tensor("enc_in", [S, D], F32R, kind="ExternalInput")
    wq1 = nc.dram_tensor("wq1", [128, NDC, 512], F32R, kind="ExternalInput")
    wk1 = nc.dram_tensor("wk1", [128, NDC, 512], F32R, kind="ExternalInput")
    wv1 = nc.dram_tensor("wv1", [128, NDC, 520], F32R, kind="ExternalInput")
    wo1 = nc.dram_tensor("wo1", [128, 4, 1024], F32R, kind="ExternalInput")
    wq2 = nc.dram_tensor("wq2", [128, NDC, 512], F32R, kind="ExternalInput")
    wk2 = nc.dram_tensor("wk2", [128, NDC, 512], F32R, kind="ExternalInput")
    wv2 = nc.dram_tensor("wv2", [128, NDC, 520], F32R, kind="ExternalInput")
    wo2 = nc.dram_tensor("wo2", [128, 4, 1024], F32R, kind="ExternalInput")
    bq1 = nc.dram_tensor("bq1", [128, 4], F32, kind="ExternalInput")
    bk1 = nc.dram_tensor("bk1", [128, 4], F32, kind="ExternalInput")
    bv1 = nc.dram_tensor("bv1", [1, 520], F32R, kind="ExternalInput")
    bo1 = nc.dram_tensor("bo1", [1, 1024], F32R, kind="ExternalInput")
    bq2 = nc.dram_tensor("bq2", [128, 4], F32, kind="ExternalInput")
    bk2 = nc.dram_tensor("bk2", [128, 4], F32, kind="ExternalInput")
    bv2 = nc.dram_tensor("bv2", [1, 520], F32R, kind="ExternalInput")
    bo2 = nc.dram_tensor("bo2", [1, 1024], F32R, kind="ExternalInput")
    selfpad = nc.dram_tensor("selfpad", [128, NKC], F32, kind="ExternalInput")
    crosspad = nc.dram_tensor("crosspad", [128, NKC], F32, kind="ExternalInput")
    onesr = nc.dram_tensor("onesr", [1, 128], F32R, kind="ExternalInput")
    eyer = nc.dram_tensor("eyer", [128, 128], F32R, kind="ExternalInput")
    eyeb = nc.dram_tensor("eyeb", [128, 128], mybir.dt.float16, kind="ExternalInput")
    eyef = nc.dram_tensor("eyef", [1, 1], F32, kind="ExternalInput")

    xo = nc.dram_tensor("xo", [T, D], F32, kind="ExternalOutput")
    ca = nc.dram_tensor("ca", [HL, T, S], F32, kind="ExternalOutput")

    cc_in = nc.dram_tensor("cc_in", [T, D], F32)
    cc_out = nc.dram_tensor("cc_out", [T, D], F32)
    groups = [[0, 1], [2, 3], [4, 5], [6, 7]]

    with tile.TileContext(nc) as tc, ExitStack() as top:
        const = top.enter_context(tc.tile_pool(name="const", bufs=1))
        t_eyer = const.tile([128, 128], F32R)
        nc.sync.dma_start(out=t_eyer, in_=eyer[:])
        t_eyeb = const.tile([128, 128], mybir.dt.float16)
        nc.sync.dma_start(out=t_eyeb, in_=eyeb[:])
        t_eyef = const.tile([1, 1], F32)
        nc.sync.dma_start(out=t_eyef, in_=eyef[:])
        t_ones = const.tile([1, 128], F32R)
        nc.sync.dma_start(out=t_ones, in_=onesr[:])
        t_spad = const.tile([128, NKC], F32)
        nc.sync.dma_start(out=t_spad, in_=selfpad[:])
        t_cpad = const.tile([128, NKC], F32)
        nc.sync.dma_start(out=t_cpad, in_=crosspad[:])
        t_eps = const.tile([128, 1], F32)
        nc.vector.memset(t_eps, EPS)
        t_bq1 = const.tile([128, 4], F32); nc.sync.dma_start(out=t_bq1, in_=bq1[:])
        t_bk1 = const.tile([128, 4], F32); nc.sync.dma_start(out=t_bk1, in_=bk1[:])
        t_bq2 = const.tile([128, 4], F32); nc.sync.dma_start(out=t_bq2, in_=bq2[:])
        t_bk2 = const.tile([128, 4], F32); nc.sync.dma_start(out=t_bk2, in_=bk2[:])
        t_bv1 = const.tile([1, 520], F32R); nc.sync.dma_start(out=t_bv1, in_=bv1[:])
        t_bo1 = const.tile([1, 1024], F32R); nc.sync.dma_start(out=t_bo1, in_=bo1[:])
        t_bv2 = const.tile([1, 520], F32R); nc.sync.dma_start(out=t_bv2, in_=bv2[:])
        t_bo2 = const.tile([1, 1024], F32R); nc.sync.dma_start(out=t_bo2, in_=bo2[:])

        xpool = top.enter_context(tc.tile_pool(name="xpool", bufs=1))
        x_t = xpool.tile([128, 8, 1024], F32)   # token = tc*128 + p
        for tc8 in range(8):
            nc.sync.dma_start(
                out=x_t[:, tc8, :],
                in_=x_in.rearrange("(t p) d -> p t d", p=128)[:, tc8, :])

        # ---------- helpers ----------
        def layernorm_chunk(lnp, psp, src_ap, dst_f32r_tile):
            """LN over free dim of a [128,1024] f32 chunk -> writes f32r tile."""
            stats = lnp.tile([128, 2, 6], F32, tag="stats")
            nc.vector.bn_stats(out=stats[:, 0], in_=src_ap[:, 0:512])
            nc.vector.bn_stats(out=stats[:, 1], in_=src_ap[:, 512:1024])
            mv = lnp.tile([128, 2], F32, tag="mv")
            nc.vector.bn_aggr(out=mv[:], in_=stats[:])
            lnv = lnp.tile([128, 1], F32, tag="lnv")
            nc.scalar.activation(out=lnv[:], in_=mv[:, 1:2], func=AF.Ln,
                                 bias=t_eps[:], scale=1.0)
            rstd = lnp.tile([128, 1], F32, tag="rstd")
            nc.scalar.activation(out=rstd[:], in_=lnv[:], func=AF.Exp,
                                 bias=0.0, scale=-0.5)
            nc.vector.tensor_scalar(out=dst_f32r_tile[:], in0=src_ap,
                                    scalar1=mv[:, 0:1], scalar2=rstd[:],
                                    op0=ALU.subtract, op1=ALU.mult)

        def transpose_chunk(psp, src_f32r_ap, dst_tile, dst_col0):
            """[128,1024] f32r normal chunk -> 8 [128,128] transposes, batched
            4 per psum bank, DVE-evicted into dst_tile[:, dc, col:col+128]."""
            for q in range(2):
                ptr = psp.tile([128, 4, 128], F32R, tag="ptr")
                for j in range(4):
                    dc = q * 4 + j
                    nc.tensor.transpose(ptr[:, j],
                                        src_f32r_ap[:, dc * 128:(dc + 1) * 128],
                                        t_eyer[:])
                nc.scalar.activation(
                    out=dst_tile[:, q * 4:(q + 1) * 4, dst_col0:dst_col0 + 128],
                    in_=ptr[:], func=AF.Copy)

        def proj_T(psp, srcT, w_dram, bias_tile, out_tile, wpool):
            """out_tile [128, 4, 1024] f32r: per hg,rcol accumulate 8 dchunks."""
            wt = wpool.tile([128, NDC, 512], F32R, tag="w")
            nc.sync.dma_start(out=wt, in_=w_dram[:])
            for hg in range(4):
                for rc2 in range(2):
                    pp = psp.tile([128, 512], F32, tag="pproj")
                    for dc in range(NDC):
                        nc.tensor.matmul(
                            pp[:], lhsT=wt[:, dc, hg * 128:(hg + 1) * 128],
                            rhs=srcT[:, dc, rc2 * 512:(rc2 + 1) * 512],
                            start=(dc == 0), stop=(dc == NDC - 1))
                    nc.vector.tensor_scalar_add(
                        out=out_tile[:, hg, rc2 * 512:(rc2 + 1) * 512], in0=pp[:],
                        scalar1=bias_tile[:, hg:hg + 1])

        def proj_V(psp, srcT, w_dram, brow_tile, out_tile, wpool):
            """V' projection: out_tile [128, NKC, 520] f32r."""
            wt = wpool.tile([128, NDC, 520], F32R, tag="w")
            nc.sync.dma_start(out=wt, in_=w_dram[:])
            for kc in range(NKC):
                for g4 in range(2):
                    pp = psp.tile([128, 260], F32, tag="pproj")
                    for dc in range(NDC):
                        nc.tensor.matmul(
                            pp[:], lhsT=srcT[:, dc, kc * 128:(kc + 1) * 128],
                            rhs=wt[:, dc, g4 * 260:(g4 + 1) * 260],
                            start=(dc == 0), stop=False)
                    nc.tensor.matmul(pp[:], lhsT=t_ones[:, 0:128],
                                     rhs=brow_tile[:, g4 * 260:(g4 + 1) * 260],
                                     start=False, stop=True)
                    nc.vector.tensor_copy(
                        out=out_tile[:, kc, g4 * 260:(g4 + 1) * 260], in_=pp[:])

        def proj_O(psp, ctxT, w_dram, brow_tile, evict_fn, wpool):
            """O projection: for each row chunk rc, produce psum [128,1024] and
            call evict_fn(rc, psum_ap)."""
            wt = wpool.tile([128, 4, 1024], F32R, tag="w")
            nc.sync.dma_start(out=wt, in_=w_dram[:])
            for rc in range(8):
                po = psp.tile([128, 2, 512], F32, tag="po")
                for nh in range(2):
                    for cc in range(4):
                        nc.tensor.matmul(
                            po[:, nh], lhsT=ctxT[:, cc, rc * 128:(rc + 1) * 128],
                            rhs=wt[:, cc, nh * 512:(nh + 1) * 512],
                            start=(cc == 0), stop=False)
                    nc.tensor.matmul(po[:, nh], lhsT=t_ones[:, 0:128],
                                     rhs=brow_tile[:, nh * 512:(nh + 1) * 512],
                                     start=False, stop=True)
                evict_fn(rc, po)

        # ---------- phase L1 + transpose, and encT ----------
        es_a = ExitStack()
        qkv1_pool = es_a.enter_context(tc.tile_pool(name="qkv1", bufs=1))
        q1T = qkv1_pool.tile([128, 4, 1024], F32R)
        k1T = qkv1_pool.tile([128, 4, 1024], F32R)
        v1 = qkv1_pool.tile([128, NKC, 520], F32R)

        es_e = ExitStack()
        encT_pool = es_e.enter_context(tc.tile_pool(name="encT", bufs=1, side="right"))
        encT = encT_pool.tile([128, NDC, S], F32R)

        mark("L1P1")
        with tc.tile_pool(name="xnT", bufs=1, side="right") as xnT_pool, \
             tc.tile_pool(name="lnp", bufs=3) as lnp, \
             tc.tile_pool(name="wp1", bufs=2) as wp1, \
             tc.tile_pool(name="psL1", bufs=4, space="PSUM") as psL1, \
             tc.tile_pool(name="psP1", bufs=3, space="PSUM") as psP1:
            xnT = xnT_pool.tile([128, NDC, T], F32R)
            for tc8 in range(8):
                xn = lnp.tile([128, 1024], F32R, tag="xn")
                layernorm_chunk(lnp, psL1, x_t[:, tc8, :], xn)
                transpose_chunk(psL1, xn[:], xnT, tc8 * 128)
            mark("encT")
            for tc8 in range(8):
                ec = lnp.tile([128, 1024], F32R, tag="xn")
                nc.sync.dma_start(
                    out=ec, in_=enc_in.rearrange("(t p) d -> p t d", p=128)[:, tc8, :])
                transpose_chunk(psL1, ec[:], encT, tc8 * 128)
            mark("P1")
            proj_T(psP1, xnT, wq1, t_bq1, q1T, wp1)
            proj_T(psP1, xnT, wk1, t_bk1, k1T, wp1)
            proj_V(psP1, xnT, wv1, t_bv1, v1, wp1)

        # ---------- phase A1: self attention ----------
        mark("A1")
        es_b = ExitStack()
        ctx1_pool = es_b.enter_context(tc.tile_pool(name="ctx1", bufs=1, side="right"))
        ctx1T = ctx1_pool.tile([128, 4, 1024], F32R)
        with tc.tile_pool(name="pt1", bufs=3) as pt1_pool, \
             tc.tile_pool(name="att_sm", bufs=2) as smp, \
             tc.tile_pool(name="psS", bufs=4, space="PSUM") as psS, \
             tc.tile_pool(name="psC", bufs=2, space="PSUM") as psC:
            for hp in range(4):
                for rb in range(NRB):
                    nkc = 4 * (rb + 1)
                    pt_a = pt1_pool.tile([128, NKC, RB], F32R, tag="pt")
                    pt_b = pt1_pool.tile([128, NKC, RB], F32R, tag="pt")
                    ptils = (pt_a, pt_b)
                    for kc in range(nkc):
                        r0 = max(0, kc * 128 - rb * RB)
                        for par in range(2):
                            po = par * 64
                            ps_s = psS.tile([128, RB], F32, tag="ps_s")
                            nc.tensor.matmul(
                                ps_s[:, r0:RB],
                                lhsT=k1T[po:po + 64, hp, kc * 128:(kc + 1) * 128],
                                rhs=q1T[po:po + 64, hp, rb * RB + r0:(rb + 1) * RB],
                                start=True, stop=True, tile_position=(po, 0))
                            nc.scalar.activation(
                                out=ptils[par][:, kc, r0:RB], in_=ps_s[:, r0:RB],
                                func=AF.Exp, bias=t_spad[:, kc:kc + 1], scale=0.125)
                            if kc >= 4 * rb:
                                nc.gpsimd.affine_select(
                                    out=ptils[par][:, kc, r0:r0 + 128],
                                    in_=ptils[par][:, kc, r0:r0 + 128],
                                    compare_op=ALU.is_ge, fill=0.0,
                                    base=0, channel_multiplier=-1,
                                    pattern=[[1, 128]])
                    for par in range(2):
                        h = 2 * hp + par
                        po = par * 64
                        ptil = ptils[par]
                        ps_c = psC.tile([65, RB], F32, tag="ps_c")
                        for kc in range(nkc):
                            r0 = max(0, kc * 128 - rb * RB)
                            nc.tensor.matmul(ps_c[:, r0:RB],
                                             lhsT=v1[:, kc, h * 65:(h + 1) * 65],
                                             rhs=ptil[:, kc, r0:RB],
                                             start=(kc == 0), stop=(kc == nkc - 1))
                        rsb = smp.tile([1, RB], F32, tag="rsb")
                        nc.vector.reciprocal(out=rsb[:], in_=ps_c[64:65, :])
                        rbc = smp.tile([64, RB], F32, tag="rbc")
                        nc.gpsimd.partition_broadcast(rbc[:], rsb[:])
                        nc.vector.tensor_mul(
                            out=ctx1T[po:po + 64, hp, rb * RB:(rb + 1) * RB],
                            in0=ps_c[0:64, :], in1=rbc[:])

        es_a.close()  # free qkv1
        mark("O1")
        # ---------- phase O1 ----------
        with tc.tile_pool(name="sa_sb", bufs=3) as sa_pool, \
             tc.tile_pool(name="wpo1", bufs=1) as wpo1, \
             tc.tile_pool(name="psO", bufs=2, space="PSUM") as psO:
            def evict_sa(rc, po):
                sa = sa_pool.tile([128, 1024], F32, tag="sa")
                nc.vector.tensor_copy(out=sa[:],
                                      in_=po[:].rearrange("p a b -> p (a b)"))
                nc.sync.dma_start(out=cc_in[rc * 128:(rc + 1) * 128, :], in_=sa[:])
            proj_O(psO, ctx1T, wo1, t_bo1, evict_sa, wpo1)
        es_b.close()  # free ctx1

        # ---------- cross projections K2/V2 (overlap the AllReduce) ----------
        mark("P2kv")
        es_c = ExitStack()
        qkv2_pool = es_c.enter_context(tc.tile_pool(name="qkv2", bufs=1))
        q2T = qkv2_pool.tile([128, 4, 1024], F32R)
        k2T = qkv2_pool.tile([128, 4, 1024], F32R)
        v2 = qkv2_pool.tile([128, NKC, 520], mybir.dt.float16)
        with tc.tile_pool(name="wp2b", bufs=2) as wp2b, \
             tc.tile_pool(name="psP2b", bufs=3, space="PSUM") as psP2b:
            proj_T(psP2b, encT, wk2, t_bk2, k2T, wp2b)
            proj_V(psP2b, encT, wv2, t_bv2, v2, wp2b)


        mark("CC")
        for q in range(4):
            if fake_cc:
                nc.sync.dma_start(out=cc_out[q * 256:(q + 1) * 256, :],
                                  in_=cc_in[q * 256:(q + 1) * 256, :])
            else:
                nc.gpsimd.collective_compute(
                    kind="AllReduce", op=mybir.AluOpType.add,
                    ins=[cc_in[q * 256:(q + 1) * 256, :]],
                    outs=[cc_out[q * 256:(q + 1) * 256, :]],
                    replica_groups=groups)

        mark("L2P2")
        with tc.tile_pool(name="xn2T", bufs=1, side="right") as xn2T_pool, \
             tc.tile_pool(name="lnp2", bufs=2) as lnp2, \
             tc.tile_pool(name="wp2", bufs=1) as wp2, \
             tc.tile_pool(name="psL2", bufs=4, space="PSUM") as psL2, \
             tc.tile_pool(name="psP2", bufs=3, space="PSUM") as psP2:
            xn2T = xn2T_pool.tile([128, NDC, T], F32R)
            for tc8 in range(8):
                sa_ch = lnp2.tile([128, 1024], F32, tag="sach")
                nc.sync.dma_start(out=sa_ch,
                                  in_=cc_out[tc8 * 128:(tc8 + 1) * 128, :])
                nc.vector.tensor_add(out=x_t[:, tc8, :], in0=x_t[:, tc8, :],
                                     in1=sa_ch[:])
                xn2 = lnp2.tile([128, 1024], F32R, tag="xn2")
                layernorm_chunk(lnp2, psL2, x_t[:, tc8, :], xn2)
                transpose_chunk(psL2, xn2[:], xn2T, tc8 * 128)
            proj_T(psP2, xn2T, wq2, t_bq2, q2T, wp2)
        es_e.close()  # free encT

        mark("A2")
        # ---------- phase A2: cross attention ----------
        es_d = ExitStack()
        ctx2_pool = es_d.enter_context(tc.tile_pool(name="ctx2", bufs=1, side="right"))
        ctx2T = ctx2_pool.tile([128, 4, 1024], F32R)
        with tc.tile_pool(name="pt2", bufs=4) as pt2_pool, \
             tc.tile_pool(name="ca_sb", bufs=3) as ca_pool, \
             tc.tile_pool(name="att_sm2", bufs=2) as smp2, \
             tc.tile_pool(name="psS2", bufs=3, space="PSUM") as psS2, \
             tc.tile_pool(name="psC2", bufs=2, space="PSUM") as psC2, \
             tc.tile_pool(name="psT2", bufs=3, space="PSUM") as psT2:
            BF16 = mybir.dt.float16
            for hp in range(4):
                for rb in range(NRB):
                    pt_a = pt2_pool.tile([128, NKC, RB], BF16, tag="pt")
                    pt_b = pt2_pool.tile([128, NKC, RB], BF16, tag="pt")
                    ptils = (pt_a, pt_b)
                    for kc in range(NKC):
                        for par in range(2):
                            po = par * 64
                            ps_s = psS2.tile([128, RB], F32, tag="ps_s")
                            nc.tensor.matmul(
                                ps_s[:],
                                lhsT=k2T[po:po + 64, hp, kc * 128:(kc + 1) * 128],
                                rhs=q2T[po:po + 64, hp, rb * RB:(rb + 1) * RB],
                                start=True, stop=True, tile_position=(po, 0))
                            nc.scalar.activation(
                                out=ptils[par][:, kc], in_=ps_s[:], func=AF.Exp,
                                bias=t_cpad[:, kc:kc + 1], scale=0.125)
                    for par in range(2):
                        h = 2 * hp + par
                        po = par * 64
                        ptil = ptils[par]
                        ps_c = psC2.tile([65, RB], F32, tag="ps_c")
                        for kc in range(NKC):
                            nc.tensor.matmul(ps_c[:],
                                             lhsT=v2[:, kc, h * 65:(h + 1) * 65],
                                             rhs=ptil[:, kc],
                                             start=(kc == 0), stop=(kc == NKC - 1))
                        rsb = smp2.tile([1, RB], F32, tag="rsb")
                        nc.vector.reciprocal(out=rsb[:], in_=ps_c[64:65, :])
                        rbc = smp2.tile([64, RB], F32, tag="rbc")
                        nc.gpsimd.partition_broadcast(rbc[:], rsb[:])
                        nc.vector.tensor_mul(
                            out=ctx2T[po:po + 64, hp, rb * RB:(rb + 1) * RB],
                            in0=ps_c[0:64, :], in1=rbc[:])
                        # per-row scale column [128, 4] via PE transpose of rsb
                        pr = psS2.tile([128, 4], F32, tag="ps_s")
                        for j in range(4):
                            nc.tensor.transpose(pr[:, j:j + 1],
                                                rsb[:, j * 128:(j + 1) * 128],
                                                t_eyef[:])
                        rcol = smp2.tile([128, 4], F32, tag="rcol")
                        nc.scalar.activation(out=rcol[:], in_=pr[:], func=AF.Copy)
                        # transpose P~ (fp16) back to [row, key], evict normalized
                        for rh in range(4):
                            casb = ca_pool.tile([128, S], F32, tag="ca")
                            for kc2 in range(2):
                                ptr2 = psT2.tile([128, 4, 128], BF16, tag="ptr2")
                                for j in range(4):
                                    kc = kc2 * 4 + j
                                    nc.tensor.transpose(
                                        ptr2[:, j],
                                        ptil[:, kc, rh * 128:(rh + 1) * 128],
                                        t_eyeb[:])
                                if (rh * 2 + kc2) % 4 == 0:
                                    nc.scalar.activation(
                                        out=casb[:, kc2 * 512:(kc2 + 1) * 512],
                                        in_=ptr2[:].rearrange("p a b -> p (a b)"),
                                        func=AF.Copy, scale=rcol[:, rh:rh + 1])
                                else:
                                    nc.vector.tensor_scalar_mul(
                                        out=casb[:, kc2 * 512:(kc2 + 1) * 512],
                                        in0=ptr2[:].rearrange("p a b -> p (a b)"),
                                        scalar1=rcol[:, rh:rh + 1])
                            nc.sync.dma_start(
                                out=ca[h, rb * RB + rh * 128: rb * RB + (rh + 1) * 128, :],
                                in_=casb[:])

        mark("O2")
        # ---------- phase O2 + output ----------
        es_c.close()  # free qkv2
        with tc.tile_pool(name="xo_sb", bufs=3) as xo_pool, \
             tc.tile_pool(name="wpo2", bufs=1) as wpo2, \
             tc.tile_pool(name="psO2", bufs=2, space="PSUM") as psO2:
            def evict_xo(rc, po):
                xov = xo_pool.tile([128, 1024], F32, tag="xo")
                nc.vector.tensor_scalar_mul(out=xov[:], in0=x_t[:, rc, :],
                                            scalar1=0.5)
                nc.vector.tensor_add(out=xov[:], in0=xov[:],
                                     in1=po[:].rearrange("p a b -> p (a b)"))
                nc.sync.dma_start(out=xo[rc * 128:(rc + 1) * 128, :], in_=xov[:])
            proj_O(psO2, ctx2T, wo2, t_bo2, evict_xo, wpo2)
        es_d.close()

    mark("end")
    nc.compile()
    bacc.get_activation_tables = _orig_tables
    return nc


def _prep_core_inputs(c, inp):
    """Host-side shard/prep for core c (b = c//2, head group g = c%2)."""
    b, g = c // 2, c % 2
    f32 = np.float32

    def fold_w(W, gvec):
        return (gvec[:, None] * W).astype(f32)

    def fold_b(W, bvec, beta):
        return (beta @ W + bvec).astype(f32)

    def wT_tiles(Wslice):          # [1024, 512] -> [128, 8, 512]
        return np.ascontiguousarray(
            Wslice.reshape(NDC, 128, Wslice.shape[1]).transpose(1, 0, 2)).astype(f32)

    def wV_tiles(Wslice, bslice):  # [1024,512],[512] -> [128,8,520],[1,520]
        Wp = np.zeros((D, HL, 65), f32)
        Wp[:, :, :64] = Wslice.reshape(D, HL, 64)
        bp = np.zeros((HL, 65), f32)
        bp[:, :64] = bslice.reshape(HL, 64)
        bp[:, 64] = 1.0
        return (np.ascontiguousarray(
                    Wp.reshape(D, 520).reshape(NDC, 128, 520).transpose(1, 0, 2)),
                bp.reshape(1, 520))

    sl = slice(g * 512, (g + 1) * 512)
    Wq1f = fold_w(inp["Wq1"], inp["ln1_g"]); bq1f = fold_b(inp["Wq1"], inp["bq1"], inp["ln1_b"])
    Wk1f = fold_w(inp["Wk1"], inp["ln1_g"]); bk1f = fold_b(inp["Wk1"], inp["bk1"], inp["ln1_b"])
    Wv1f = fold_w(inp["Wv1"], inp["ln1_g"]); bv1f = fold_b(inp["Wv1"], inp["bv1"], inp["ln1_b"])
    Wq2f = fold_w(inp["Wq2"], inp["ln2_g"]); bq2f = fold_b(inp["Wq2"], inp["bq2"], inp["ln2_b"])
    Wk2f = inp["Wk2"].astype(f32); bk2f = inp["bk2"].astype(f32)
    Wv2f = inp["Wv2"].astype(f32); bv2f = inp["bv2"].astype(f32)

    wv1t, bv1r = wV_tiles(Wv1f[:, sl], bv1f[sl])
    wv2t, bv2r = wV_tiles(Wv2f[:, sl], bv2f[sl])

    karr = np.arange(S).reshape(NKC, 128).T  # [128, NKC] global key index
    spad = np.where(karr < int(inp["tgt_lengths"][b]), 0.0, NEG).astype(f32)
    cpad = np.where(karr < int(inp["src_lengths"][b]), 0.0, NEG).astype(f32)

    return {
        "x_in": np.ascontiguousarray(inp["x"][b]).astype(f32),
        "enc_in": np.ascontiguousarray(inp["encoder_output"][b]).astype(f32),
        "wq1": wT_tiles(Wq1f[:, sl]), "wk1": wT_tiles(Wk1f[:, sl]), "wv1": wv1t,
        "wo1": np.ascontiguousarray(
            inp["Wo1"][sl, :].reshape(4, 128, 1024).transpose(1, 0, 2)).astype(f32),
        "wq2": wT_tiles(Wq2f[:, sl]), "wk2": wT_tiles(Wk2f[:, sl]), "wv2": wv2t,
        "wo2": np.ascontiguousarray(
            inp["Wo2"][sl, :].reshape(4, 128, 1024).transpose(1, 0, 2)).astype(f32),
        "bq1": bq1f[sl].reshape(4, 128).T.copy(),
        "bk1": bk1f[sl].reshape(4, 128).T.copy(),
        "bv1": bv1r, "bo1": (0.5 * inp["bo1"]).astype(f32).reshape(1, 1024),
        "bq2": bq2f[sl].reshape(4, 128).T.copy(),
        "bk2": bk2f[sl].reshape(4, 128).T.copy(),
        "bv2": bv2r, "bo2": (0.5 * inp["bo2"]).astype(f32).reshape(1, 1024),
        "selfpad": spad, "crosspad": cpad,
        "onesr": np.ones((1, 128), f32),
        "eyer": np.eye(128, dtype=f32),
        "eyeb": np.eye(128, dtype=np.float16),
        "eyef": np.ones((1, 1), f32),
    }


def kernel(**inputs):
    global _PROG
    from concourse.bass_utils import run_bass_kernel_spmd

    inp = {k: np.asarray(v) for k, v in inputs.items()}
    if _PROG is None:
        _PROG = _build_program()

    in_maps = [_prep_core_inputs(c, inp) for c in range(8)]
    res = run_bass_kernel_spmd(_PROG, in_maps, core_ids=list(range(8))).results

    x_out = np.empty((B, T, D), np.float32)
    ca_w = np.empty((B, H, T, S), np.float32)
    for b in range(B):
        x_out[b] = res[2 * b]["xo"] + res[2 * b + 1]["xo"]
        ca_w[b, 0:HL] = res[2 * b]["ca"]
        ca_w[b, HL:H] = res[2 * b + 1]["ca"]
    return x_out, ca_w
